# revision 12
# baseline (speedup 1.0000x reference)
"""Trainium2 Bass kernel for a single-head BERT attention (B=8, S=2048, E=1024, H=64).

Sharding: data-parallel over batch — one batch element per NeuronCore (8 cores).
Weights replicated. No collectives.

Per-core layout (all matmuls bf16 with fp32 PSUM accumulation):
  qkT  = [Wq|Wk]^T-projection: out [128, S]  (rows 0-63 = q^T, 64-127 = k^T)
  v    = natural [S, H] per 128-row tile, with an appended ones column so the
         second matmul produces the softmax denominator for free
  wT   = scoresT[t, s] = k^T.T @ q^T  (contract over H=64)
  pT   = exp(wT / 8) (ScalarE), multiplied by the (host-transposed) mask only on
         128x128 blocks that are mixed; all-zero blocks are skipped entirely
  ctx  = [s, h|denom] = pT.T @ [v|1], then rows normalized by 1/denom

The program is specialized at runtime from the actual mask contents (block map of
all-zero / all-one / mixed 128x128 blocks, reduced over the batch), so any mask is
handled correctly; for the causal mask this halves the score/exp/context work.
The v / scores / exp / context work is interleaved per t-tile so LDWEIGHTS traffic
hides under matmul streaming and ScalarE overlaps the tensor engine.
"""

import numpy as np
import ml_dtypes

import concourse.bass as bass  # noqa: F401  (import registers bass machinery)
import concourse.bacc as bacc
import concourse.mybir as mybir
import concourse.tile as tile
from concourse.bass_utils import run_bass_kernel_spmd

BF16 = ml_dtypes.bfloat16
B, S, E, H = 8, 2048, 1024, 64
P = 128          # partitions / tile edge
NS = S // P      # 16 seq tiles
NE = E // P      # 8 embed chunks
SB = 512         # mm1 max s-block (one fp32 PSUM bank)
NSB = S // SB

_cache: dict = {}
last_results = None  # BassKernelResults of the most recent run (for test harness)


def _plan_from_mask(mask: np.ndarray):
    """Derive the static block plan from the actual mask input.

    Returns (ranges, mask_items, mm2_lists, maskT):
      ranges[j]    = (lo, hi) element range of s that t-tile j must compute (or None)
      mask_items   = [(j, i)] 128x128 blocks needing an elementwise mask multiply
      mm2_lists[i] = t-tiles contributing to output s-tile i
    Valid for every batch element simultaneously (classifications reduced over batch).
    """
    m = np.asarray(mask, dtype=bool)
    mt = np.ascontiguousarray(m.transpose(0, 2, 1))  # [B, t, s]
    blocks = mt.reshape(B, NS, P, NS, P)
    any_ = blocks.any(axis=(2, 4))   # [B, tj, si]
    all_ = blocks.all(axis=(2, 4))
    nz = any_.any(axis=0)            # not all-zero in some batch -> must compute
    allone = all_.all(axis=0)        # all-ones in every batch -> no mask needed
    mixed = nz & ~allone

    ranges = []
    for j in range(NS):
        cols = np.nonzero(nz[j])[0]
        if len(cols) == 0:
            ranges.append(None)
            continue
        ranges.append((int(cols.min()) * P, (int(cols.max()) + 1) * P))

    mask_items = [(j, i) for j in range(NS) for i in range(NS) if mixed[j, i]]
    mm2 = [tuple(int(j) for j in np.nonzero(nz[:, i])[0]) for i in range(NS)]
    return ranges, mask_items, mm2, mt


def _build_nc(ranges, mask_items, mm2, has_bqk, has_bv):
    dt = mybir.dt
    n_mb = max(len(mask_items), 1)
    nc = bacc.Bacc("TRN2", target_bir_lowering=False, debug=False, num_devices=8)

    xT_d = nc.dram_tensor("xT", [E, S], dt.bfloat16, kind="ExternalInput").ap()
    wqk_d = nc.dram_tensor("wqk", [P, NE * 2 * H], dt.bfloat16, kind="ExternalInput").ap()
    wv_d = nc.dram_tensor("wv", [P, NE * H], dt.bfloat16, kind="ExternalInput").ap()
    bqk_d = nc.dram_tensor("bqk", [1, 2 * H], dt.bfloat16, kind="ExternalInput").ap()
    bv_d = nc.dram_tensor("bv", [1, H], dt.bfloat16, kind="ExternalInput").ap()
    mb_d = nc.dram_tensor("maskb", [P, n_mb * P], dt.bfloat16, kind="ExternalInput").ap()
    y_d = nc.dram_tensor("y", [S, H], dt.float32, kind="ExternalOutput").ap()

    EXP = mybir.ActivationFunctionType.Exp
    with tile.TileContext(nc) as tc:
        with (
            tc.tile_pool(name="consts", bufs=1) as cpool,
            tc.tile_pool(name="xt", bufs=1) as xpool,
            tc.tile_pool(name="qk", bufs=1) as qkpool,
            tc.tile_pool(name="vex", bufs=1) as vpool,
            tc.tile_pool(name="pt", bufs=1) as ppool,
            tc.tile_pool(name="maskp", bufs=1) as mpool,
            tc.tile_pool(name="outs", bufs=4) as opool,
            tc.tile_pool(name="wps", bufs=3, space="PSUM") as wpsum,
            tc.tile_pool(name="sm", bufs=2, space="PSUM") as smpsum,
        ):
            # ---- constants ----
            wqk_sb = cpool.tile([P, NE, 2 * H], dt.bfloat16)
            nc.sync.dma_start(wqk_sb[:], wqk_d.rearrange("p (c h) -> p c h", c=NE))
            wv_sb = cpool.tile([P, NE, H], dt.bfloat16)
            nc.sync.dma_start(wv_sb[:], wv_d.rearrange("p (c h) -> p c h", c=NE))
            bqk_sb = cpool.tile([1, 2 * H], dt.bfloat16)
            nc.sync.dma_start(bqk_sb[:], bqk_d[:])
            bv_sb = cpool.tile([1, H], dt.bfloat16)
            nc.sync.dma_start(bv_sb[:], bv_d[:])
            ones_sb = cpool.tile([1, SB], dt.bfloat16)
            nc.vector.memset(ones_sb[:], 1.0)
            warm_sb = cpool.tile([1, 2], dt.float32)
            nc.scalar.activation(warm_sb[:], ones_sb[0:1, 0:2], EXP, scale=0.125)

            # ---- x^T tiles (one per 128-row embed chunk) ----
            xt = []
            for c in range(NE):
                t = xpool.tile([P, S], dt.bfloat16, tag=f"xt{c}", name=f"xt{c}")
                nc.sync.dma_start(t[:], xT_d[c * P:(c + 1) * P, :])
                xt.append(t)

            # ---- mask blocks: one packed DMA, sliced per block ----
            mask_all = mpool.tile([P, n_mb * P], dt.bfloat16, name="mask_all")
            nc.gpsimd.dma_start(mask_all[:], mb_d[:])
            mask_tiles = {}
            for idx, (j, i) in enumerate(mask_items):
                mask_tiles[(j, i)] = mask_all[:, idx * P:(idx + 1) * P]

            # ---- q/k projection: qkT[0:64] = q^T, qkT[64:128] = k^T ----
            qkT_sb = qkpool.tile([P, S], dt.bfloat16)
            kT_sb = qkpool.tile([64, S], dt.bfloat16)
            hp = tc.high_priority()
            hp.__enter__()
            qk_big = [wpsum.tile([P, 2 * SB], dt.float32, tag="wps", name="qkps")
                      for _ in range(NSB // 2)]
            qk_ps = [qk_big[i // 2][:, (i % 2) * SB:(i % 2 + 1) * SB] for i in range(NSB)]
            fix_eng = [nc.sync, nc.gpsimd]
            for c in range(NE):
                for sb_i in range(NSB):
                    blk = slice(sb_i * SB, (sb_i + 1) * SB)
                    nc.tensor.matmul(
                        qk_ps[sb_i], wqk_sb[:, c, :], xt[c][:, blk],
                        start=(c == 0), stop=(not has_bqk and c == NE - 1))
                    if c == NE - 1:
                        if has_bqk:
                            nc.tensor.matmul(qk_ps[sb_i], bqk_sb[:], ones_sb[:],
                                             start=False, stop=True)
                        nc.vector.tensor_copy(qkT_sb[:, blk], qk_ps[sb_i])
                        # partition fixup: k^T of this block down to partitions 0-63
                        fix_eng[sb_i % 2].dma_start(kT_sb[:, blk], qkT_sb[64:128, blk])

            hp.__exit__(None, None, None)

            # ---- interleaved main loop over t-tiles ----
            vext = [vpool.tile([P, H + 1], dt.bfloat16, tag=f"vx{j}", name=f"vx{j}")
                    for j in range(NS)]
            pt = [ppool.tile([P, S], dt.bfloat16, tag=f"pt{j}", name=f"pt{j}")
                  for j in range(NS)]

            def emit_v(j):
                vt = vext[j]
                nc.vector.memset(vt[:, H:H + 1], 1.0)
                pv = smpsum.tile([P, H + 1], dt.float32, tag="sm", name="pv")
                for c in range(NE):
                    nc.tensor.matmul(
                        pv[:, 0:H], xt[c][:, j * P:(j + 1) * P], wv_sb[:, c, :],
                        start=(c == 0), stop=(not has_bv and c == NE - 1))
                if has_bv:
                    nc.tensor.matmul(pv[:, 0:H], ones_sb[:, 0:P], bv_sb[:], start=False, stop=True)
                nc.vector.tensor_copy(vt[:, 0:H], pv[:, 0:H])

            def emit_mm1(j):
                lo, hi = ranges[j]
                off = lo
                while off < hi:
                    w = min(2 * SB, hi - off)
                    ps = wpsum.tile([P, 2 * SB], dt.float32, tag="wps", name="wps")
                    for o2 in range(0, w, SB):
                        w2 = min(SB, w - o2)
                        nc.tensor.matmul(
                            ps[:, o2:o2 + w2], kT_sb[:, j * P:(j + 1) * P],
                            qkT_sb[0:64, off + o2:off + o2 + w2],
                            start=True, stop=True, tile_position=(0, 0))
                    nc.scalar.activation(pt[j][:, off:off + w], ps[:, 0:w], EXP, scale=0.125)
                    off += w
                for i in range(lo // P, hi // P):
                    if (j, i) in mask_tiles:
                        sl = pt[j][:, i * P:(i + 1) * P]
                        nc.vector.tensor_mul(sl, sl, mask_tiles[(j, i)])

            y_g = y_d.rearrange("(g i p) h -> g p i h", i=4, p=P)
            obufs = {}

            def emit_mm2(i):
                g, ii = divmod(i, 4)
                if g not in obufs:
                    obufs[g] = opool.tile([P, 4, H], dt.float32, tag="out",
                                          name=f"ob{g}")
                ob = obufs[g][:, ii, :]
                js = mm2[i]
                if not js:
                    nc.vector.memset(ob, 0.0)
                else:
                    pc = smpsum.tile([P, H + 1], dt.float32, tag="sm", name="pc")
                    for n, j in enumerate(js):
                        nc.tensor.matmul(
                            pc[:], pt[j][:, i * P:(i + 1) * P], vext[j][:],
                            start=(n == 0), stop=(n == len(js) - 1))
                    rc = opool.tile([P, 1], dt.float32, tag="recip", name="rc")
                    nc.vector.reciprocal(rc[:], pc[:, H:H + 1])
                    nc.vector.tensor_scalar_mul(ob, pc[:, 0:H], rc[:])
                if ii == 3:
                    nc.gpsimd.dma_start(y_g[g], obufs[g][:])

            for j in range(NS):
                if ranges[j] is not None:
                    emit_mm1(j)
                emit_v(j)
                # mm2 for s-tile j only needs pT[j' <= j] under a causal-like
                # plan; emit as soon as every dependency has been emitted.
                if all(jj <= j for jj in mm2[j]):
                    emit_mm2(j)
            for i in range(NS):
                if not all(jj <= i for jj in mm2[i]):
                    emit_mm2(i)

    nc.compile()
    return nc


def kernel(x, mask, Wq, bq, Wk, bk, Wv, bv, _trace=False, _trace_kwargs=None):
    global last_results
    x = np.asarray(x, dtype=np.float32)
    ranges, mask_items, mm2, maskT = _plan_from_mask(mask)

    has_bqk = bool(np.any(bq)) or bool(np.any(bk))
    has_bv = bool(np.any(bv))
    key = (tuple(ranges), tuple(mask_items), tuple(mm2), has_bqk, has_bv)
    nc = _cache.get(key)
    if nc is None:
        nc = _build_nc(ranges, mask_items, mm2, has_bqk, has_bv)
        _cache[key] = nc

    wqk = np.concatenate([np.asarray(Wq), np.asarray(Wk)], axis=1)
    wqk = np.ascontiguousarray(
        wqk.reshape(NE, P, 2 * H).transpose(1, 0, 2)).reshape(P, NE * 2 * H).astype(BF16)
    wv = np.ascontiguousarray(
        np.asarray(Wv).reshape(NE, P, H).transpose(1, 0, 2)).reshape(P, NE * H).astype(BF16)
    bqk = np.concatenate([np.asarray(bq), np.asarray(bk)])[None, :].astype(BF16)
    bvv = np.asarray(bv)[None, :].astype(BF16)

    in_maps = []
    for b in range(B):
        xT_b = np.ascontiguousarray(x[b].T).astype(BF16)
        if mask_items:
            mb = np.concatenate([
                maskT[b, j * P:(j + 1) * P, i * P:(i + 1) * P]
                for (j, i) in mask_items], axis=1).astype(BF16)
        else:
            mb = np.zeros((P, P), dtype=BF16)
        in_maps.append({
            "xT": xT_b, "wqk": wqk, "wv": wv, "bqk": bqk, "bv": bvv, "maskb": mb,
        })

    res = run_bass_kernel_spmd(
        nc, in_maps, core_ids=list(range(B)),
        trace=_trace, **(_trace_kwargs or {}))
    last_results = res
    return np.stack([res.results[b]["y"] for b in range(B)])
